# revision 1
# baseline (speedup 1.0000x reference)
"""Distributed Trainium2 Bass kernel for multi-head attention w/ RoPE.

Reference op (B=4, S=2048, D=1024, H=16, HD=64, fp32):
    q/k/v = hidden @ W{q,k,v}.T + b   (per-head reshape)
    q, k  = rope(q), rope(k)
    out   = softmax(q k^T / sqrt(HD)) v  @ Wo.T

Sharding: 8 cores = 4 batches x 2 query-halves. Each core computes the
K/V projections for its whole batch (duplicated across the half-pair --
this avoids every collective), Q projection + attention for its own 1024
queries, and the o-projection for its own output rows. Host-side unshard
is a pure concat. Per-core x^T is column-permuted so the core's own
queries always sit at columns 0:1024 (keeps the SPMD graph uniform);
K/V/rope tables follow the same permutation, which softmax+V is
invariant to.

Single fused pipeline, fully transposed layout (features on partitions):
V projects first (natural layout, ones column appended so the softmax
denominator falls out of the attn@V matmul); then per head-pair: Q^T/K^T
projection chunks -> RoPE (DVE muls + a batched DMA partition band-swap
+ one 2x bf16 add) -> scores S^T with even/odd heads issued on disjoint
PE row-groups (64-row tiling, concurrent) -> wide [128,1024] exp on ACT
with the 1/sqrt(HD) scale folded in -> attn@V accumulation interleaved
at k-chunk granularity so PE fills ACT's exp latency. Normalization is
evicted-early (DVE copy frees PSUM), exact reciprocal runs base-aligned,
gpsimd hops/broadcasts it, and odd heads write via a DMA partition hop.
The o-projection consumes the transposed attention output directly.
Nonzero biases ride an augmented K=1 contraction row (skipped when the
caller's biases are all zero). All matmuls bf16, fp32 accumulation.
"""

import sys

import numpy as np

try:  # concourse ships in the container; fall back to the staged repo
    import concourse.bass  # noqa: F401
except Exception:  # pragma: no cover
    sys.path.insert(0, "/opt/trn_rl_repo")

import ml_dtypes

B, S, D, H = 4, 2048, 1024, 16
HD = D // H                      # 64
P = 128
NCORES = 8
SQ = S // 2                      # 1024 queries per core
SK = S                           # 2048 keys per core
ND = D // P                      # 8 feature chunks
NT = SK // P                     # 16 key/token chunks
QF = 512                         # matmul moving width
NQF = SQ // QF                   # 2
ROPE_BASE = 10000.0
BF16 = ml_dtypes.bfloat16

TRACE = False                    # test harness flips this
TRACE_KW = {}
LAST = {}                        # exec_time_ns / trace path for test harness

_cache = {}


def _build_nc(with_bias):
    import concourse.bass as bass
    import concourse.mybir as mybir
    import concourse.tile as tile
    from concourse import bacc
    from contextlib import ExitStack

    f32 = mybir.dt.float32
    bf16 = mybir.dt.bfloat16
    AF = mybir.ActivationFunctionType
    PSUM = bass.MemorySpace.PSUM

    nc = bacc.Bacc(None)
    xT = nc.declare_dram_parameter("xT", [D + 1, SK], bf16, False)
    wqT = nc.declare_dram_parameter("wqT", [D + 1, D], bf16, False)
    wkT = nc.declare_dram_parameter("wkT", [D + 1, D], bf16, False)
    wvT = nc.declare_dram_parameter("wvT", [D + 1, D], bf16, False)
    woT = nc.declare_dram_parameter("woT", [D, D], bf16, False)
    cosk = nc.declare_dram_parameter("cosk", [P, SK], bf16, False)
    sink = nc.declare_dram_parameter("sink", [P, SK], bf16, False)
    out = nc.declare_dram_parameter("out", [SQ, D], f32, True)

    with tile.TileContext(nc) as tc, ExitStack() as st:
        sb = st.enter_context(tc.tile_pool(name="sb", bufs=1))
        qk = st.enter_context(tc.tile_pool(name="qk", bufs=3))
        wp = st.enter_context(tc.tile_pool(name="wp", bufs=2))
        tp = st.enter_context(tc.tile_pool(name="tp", bufs=2))
        etp = st.enter_context(tc.tile_pool(name="et", bufs=6))
        npool = st.enter_context(tc.tile_pool(name="nrm", bufs=3))
        outp = st.enter_context(tc.tile_pool(name="ou", bufs=2))
        psp = st.enter_context(tc.tile_pool(name="ps", bufs=2, space=PSUM))

        vst = [sb.tile([P, H, HD + 1], bf16, tag=f"v{t}", name=f"v{t}")
               for t in range(NT)]
        at = [sb.tile([P, SQ], bf16, tag=f"at{i}", name=f"at{i}")
              for i in range(ND)]

        # ---- loads -----------------------------------------------------
        wv = wp.tile([P, ND, D], bf16, tag="wbig", name="wv", bufs=1)
        for d_ in range(ND):
            nc.sync.dma_start(out=wv[:, d_, :], in_=wvT[d_ * P:(d_ + 1) * P, :])
        xs = [sb.tile([P, SK], bf16, tag=f"x{d}", name=f"x{d}")
              for d in range(ND)]
        for d_ in range(ND):
            nc.sync.dma_start(out=xs[d_][:], in_=xT[d_ * P:(d_ + 1) * P, :])
        ck = sb.tile([P, SK], bf16, tag="ck", name="ck")
        sk_ = sb.tile([P, SK], bf16, tag="sk", name="sk")
        nc.sync.dma_start(out=ck[:], in_=cosk[:, :])
        nc.sync.dma_start(out=sk_[:], in_=sink[:, :])
        if with_bias:
            xone = sb.tile([1, SK], bf16, tag="xone", name="xone")
            nc.sync.dma_start(out=xone[:], in_=xT[D:D + 1, :])
            wvb = wp.tile([1, D], bf16, tag="wvb", name="wvb", bufs=1)
            nc.sync.dma_start(out=wvb[:], in_=wvT[D:D + 1, :])

        # ---- V projection (natural layout, x^T stationary) -------------
        for t_ in range(NT):
            for oh in range(2):
                ps = psp.tile([P, QF], f32, tag="pp", name="pp")
                for d_ in range(ND):
                    nc.tensor.matmul(
                        ps[:], xs[d_][:, t_ * P:(t_ + 1) * P],
                        wv[:, d_, oh * QF:(oh + 1) * QF],
                        start=(d_ == 0), stop=(not with_bias and d_ == ND - 1))
                if with_bias:
                    nc.tensor.matmul(
                        ps[:], xone[:, t_ * P:(t_ + 1) * P],
                        wvb[:, oh * QF:(oh + 1) * QF],
                        start=False, stop=True)
                nc.scalar.activation(
                    vst[t_][:, oh * 8:(oh + 1) * 8, 0:HD],
                    ps[:].rearrange("p (h d) -> p h d", d=HD), AF.Copy)
            nc.vector.memset(vst[t_][:, :, HD:HD + 1], 1.0)

        def load_wslice(wdram, wtag):
            ws = wp.tile([P, ND, P], bf16, tag=wtag, name=wtag)
            nc.sync.dma_start(
                out=ws[:],
                in_=wdram[0:D, :].rearrange("(n p) o -> p n o", p=P))
            wb = None
            if with_bias:
                wb = wp.tile([1, P], bf16, tag=wtag + "b", name=wtag + "b")
                nc.sync.dma_start(out=wb[:], in_=wdram[D:D + 1, :])
            return ws, wb

        def qk_proj(wsb, dst, ntok):
            """dst[o128, t] = rope(W[pi-slice] @ x^T + b); rope swap+add
            runs per 1024-wide half so scores can start on half 0. Q's
            rope table is the leading [*, 0:SQ] slice of K's (own tokens
            first in the x^T perm)."""
            ws, wb = wsb
            t2 = tp.tile([P, ntok], bf16, tag="t2", name="t2")
            t2s = tp.tile([P, ntok], bf16, tag="t2s", name="t2s")
            for c in range(ntok // QF):
                ps = psp.tile([P, QF], f32, tag="pp", name="pp")
                for d_ in range(ND):
                    nc.tensor.matmul(
                        ps[:], ws[:, d_, :], xs[d_][:, c * QF:(c + 1) * QF],
                        start=(d_ == 0), stop=(not with_bias and d_ == ND - 1))
                if with_bias:
                    nc.tensor.matmul(
                        ps[:], wb[:], xone[:, c * QF:(c + 1) * QF],
                        start=False, stop=True)
                cslice = slice(c * QF, (c + 1) * QF)
                nc.vector.tensor_mul(dst[:, cslice], ps[:], ck[:, cslice])
                nc.vector.tensor_mul(t2[:, cslice], ps[:], sk_[:, cslice])
                if c % 2 == 1:
                    # band swap d<->d+32 (scalar engine's DMA queue) + add,
                    # batched over the finished 1024-wide half
                    hs_ = slice((c - 1) * QF, (c + 1) * QF)
                    for b0 in (0, 64):
                        nc.scalar.dma_start(
                            out=t2s[b0:b0 + 32, hs_], in_=t2[b0 + 32:b0 + 64, hs_])
                        nc.scalar.dma_start(
                            out=t2s[b0 + 32:b0 + 64, hs_], in_=t2[b0:b0 + 32, hs_])
                    nc.vector.tensor_add(
                        dst[:, hs_], dst[:, hs_], t2s[:, hs_])

        # ---- fused per-head-pair projection + attention ----------------
        pending = []

        def flush_norm():
            # normalize in SBUF: exact reciprocal base-aligned at p64,
            # gpsimd DMA-hop to p0, partition-broadcast, multiply
            for h, ppi, qqs, osb in pending:
                # spread the 512 sums across 64 partitions so the exact
                # reciprocal runs 64-wide (~0.3us) instead of single-lane
                # (3.3us), keeping the DVE FIFO clear for PSUM evicts
                smr = npool.tile([HD, 8], f32, tag="smr", name="smr")
                nc.gpsimd.dma_start(out=smr[:], in_=osb[HD:HD + 1, :])
                rcs = npool.tile([HD, 8], f32, tag="rcs", name="rcs")
                nc.vector.reciprocal(rcs[:], smr[:])
                rc = npool.tile([1, QF], f32, tag="rc", name="rc")
                nc.gpsimd.dma_start(out=rc[:], in_=rcs[:])
                bc = npool.tile([HD, QF], f32, tag="bc", name="bc")
                nc.gpsimd.partition_broadcast(bc[:], rc[:])
                if h % 2 == 0:
                    nc.vector.tensor_mul(
                        at[ppi][0:64, qqs], osb[0:HD, :], bc[:])
                else:
                    # odd heads land at partition base 64; a pure-SBUF
                    # base-shifted DVE write corrupts, so write at base
                    # 0 and DMA-hop into place
                    atm = npool.tile([HD, QF], bf16, tag="atm", name="atm")
                    nc.vector.tensor_mul(atm[:], osb[0:HD, :], bc[:])
                    nc.gpsimd.dma_start(out=at[ppi][64:128, qqs], in_=atm[:])
            pending.clear()

        wnext = (load_wslice(wqT[:, 0:P], "wq"), load_wslice(wkT[:, 0:P], "wk"))
        for pi in range(ND):
            wcur = wnext
            if pi + 1 < ND:
                osl = slice((pi + 1) * P, (pi + 2) * P)
                wnext = (load_wslice(wqT[:, osl], "wq"),
                         load_wslice(wkT[:, osl], "wk"))
            qtile = qk.tile([P, SQ], bf16, tag="qt", name="qt")
            qk_proj(wcur[0], qtile, SQ)
            ktile = qk.tile([P, SK], bf16, tag="kt", name="kt")
            qk_proj(wcur[1], ktile, SK)
            flush_norm()

            for qh in range(NQF):
                qs = slice(qh * QF, (qh + 1) * QF)
                ope = psp.tile([HD + 1, QF], f32, tag="o", name="o")
                opo = psp.tile([HD + 1, QF], f32, tag="o", name="o")
                prev = None
                for kcp in range(NT // 2):
                    # even/odd heads on disjoint PE row groups: the T0/T8
                    # pairs execute concurrently in 64-row tiling mode
                    spe = psp.tile([P, 2 * QF], f32, tag="s", name="s")
                    spo = psp.tile([P, 2 * QF], f32, tag="s", name="s")
                    for j in range(2):
                        ks_ = slice((2 * kcp + j) * P, (2 * kcp + j + 1) * P)
                        js = slice(j * QF, (j + 1) * QF)
                        nc.tensor.matmul(
                            spe[:, js], ktile[0:64, ks_], qtile[0:64, qs],
                            start=True, stop=True)
                        nc.tensor.matmul(
                            spo[:, js], ktile[64:128, ks_], qtile[64:128, qs],
                            start=True, stop=True)
                    ee = etp.tile([P, 2 * QF], bf16, tag="e", name="e")
                    eo = etp.tile([P, 2 * QF], bf16, tag="e", name="e")
                    nc.scalar.activation(ee[:], spe[:], AF.Exp, scale=0.125)
                    nc.scalar.activation(eo[:], spo[:], AF.Exp, scale=0.125)
                    # attn@V for the previous k-chunk pair overlaps this
                    # pair's exp latency on the PE
                    if prev is not None:
                        pee, peo, pk = prev
                        for j in range(2):
                            kc = 2 * pk + j
                            js = slice(j * QF, (j + 1) * QF)
                            nc.tensor.matmul(
                                ope[:], vst[kc][:, 2 * pi, :], pee[:, js],
                                start=(kc == 0), stop=False)
                            nc.tensor.matmul(
                                opo[:], vst[kc][:, 2 * pi + 1, :], peo[:, js],
                                start=(kc == 0), stop=False)
                    prev = (ee, eo, kcp)
                pee, peo, pk = prev
                for j in range(2):
                    kc = 2 * pk + j
                    js = slice(j * QF, (j + 1) * QF)
                    nc.tensor.matmul(
                        ope[:], vst[kc][:, 2 * pi, :], pee[:, js],
                        start=False, stop=(kc == NT - 1))
                    nc.tensor.matmul(
                        opo[:], vst[kc][:, 2 * pi + 1, :], peo[:, js],
                        start=False, stop=(kc == NT - 1))

                for h, op in ((2 * pi, ope), (2 * pi + 1, opo)):
                    # evict PSUM immediately (quick DVE copy frees the "o"
                    # slot); the reciprocal chain is emitted one head-pair
                    # later so its 3.3us DVE reciprocals execute while DVE
                    # is otherwise idle and never block PE's PSUM evicts
                    osb = npool.tile([HD + 1, QF], f32, tag="osb", name="osb",
                                     bufs=10)
                    nc.vector.tensor_copy(osb[:], op[:])
                    pending.append((h, pi, qs, osb))

        flush_norm()

        # ---- o-projection ---------------------------------------------
        wo = wp.tile([P, ND, D], bf16, tag="wbig", name="wo", bufs=1)
        for d_ in range(ND):
            nc.sync.dma_start(out=wo[:, d_, :], in_=woT[d_ * P:(d_ + 1) * P, :])
        for qc in range(ND):
            for oh in range(2):
                ps = psp.tile([P, QF], f32, tag="pp", name="pp")
                for f in range(ND):
                    nc.tensor.matmul(
                        ps[:], at[f][:, qc * P:(qc + 1) * P],
                        wo[:, f, oh * QF:(oh + 1) * QF],
                        start=(f == 0), stop=(f == ND - 1))
                ob = outp.tile([P, QF], f32, tag="ob", name="ob")
                nc.scalar.activation(ob[:], ps[:], AF.Copy)
                nc.sync.dma_start(
                    out=out[qc * P:(qc + 1) * P, oh * QF:(oh + 1) * QF],
                    in_=ob[:])
    nc.compile()
    return nc


def _rope_tables(pos):
    """pos [n] -> (cos [128, n] bf16, sign-folded sin [128, n] bf16)."""
    inv = ROPE_BASE ** (-np.arange(0, HD, 2, dtype=np.float64) / HD)
    fr = np.outer(pos.astype(np.float64), inv)          # [n, 32]
    c, s = np.cos(fr), np.sin(fr)
    cos64 = np.concatenate([c, c], axis=1).T            # [64, n]
    sinA = np.concatenate([s, -s], axis=1).T            # [64, n]
    return (np.tile(cos64, (2, 1)).astype(BF16),
            np.tile(sinA, (2, 1)).astype(BF16))


def _aug_w(w, b):
    """[D, D] weight + [D] bias -> bf16 [D+1, D] (W.T with bias row)."""
    wa = np.empty((D + 1, D), dtype=np.float32)
    wa[:D] = np.asarray(w, dtype=np.float32).T
    wa[D] = np.asarray(b, dtype=np.float32)
    return np.ascontiguousarray(wa).astype(BF16)


def kernel(hidden_states, position_ids, Wq, bq, Wk, bk, Wv, bv, Wo):
    from concourse import bass_utils

    with_bias = bool(
        np.any(np.asarray(bq)) or np.any(np.asarray(bk)) or np.any(np.asarray(bv)))
    key = ("nc", with_bias)
    if key not in _cache:
        _cache[key] = _build_nc(with_bias)
    nc = _cache[key]

    hs = np.asarray(hidden_states, dtype=np.float32)
    pos = np.asarray(position_ids)
    wq = _aug_w(Wq, bq)
    wk = _aug_w(Wk, bk)
    wv = _aug_w(Wv, bv)
    wo = np.ascontiguousarray(np.asarray(Wo, dtype=np.float32).T).astype(BF16)

    in_maps = []
    for core in range(NCORES):
        b, hf = core // 2, core % 2
        perm = np.concatenate([
            np.arange(hf * SQ, (hf + 1) * SQ),
            np.arange((1 - hf) * SQ, (2 - hf) * SQ)])
        xp = hs[b][perm]                                 # [S, D], own half first
        xT = np.empty((D + 1, SK), dtype=np.float32)
        xT[:D] = xp.T
        xT[D] = 1.0
        ck, sk = _rope_tables(np.asarray(pos[b][perm]))
        in_maps.append({
            "xT": xT.astype(BF16), "wqT": wq, "wkT": wk, "wvT": wv, "woT": wo,
            "cosk": ck, "sink": sk,
        })

    res = bass_utils.run_bass_kernel_spmd(
        nc, in_maps, core_ids=list(range(NCORES)), trace=TRACE, **TRACE_KW)
    LAST["exec_time_ns"] = res.exec_time_ns
    LAST["mean_exec_time_ns"] = res.mean_exec_time_ns
    LAST["trace"] = res.instructions_and_trace
    LAST["profile_json"] = res.profile_json

    outp_full = np.empty((B, S, D), dtype=np.float32)
    for core in range(NCORES):
        b, hf = core // 2, core % 2
        outp_full[b, hf * SQ:(hf + 1) * SQ] = res.results[core]["out"]
    return outp_full



# revision 2
# speedup vs baseline: 1.4086x; 1.4086x over previous
"""Distributed Trainium2 Bass kernel for multi-head attention w/ RoPE.

Reference op (B=4, S=2048, D=1024, H=16, HD=64, fp32):
    q/k/v = hidden @ W{q,k,v}.T + b   (per-head reshape)
    q, k  = rope(q), rope(k)
    out   = softmax(q k^T / sqrt(HD)) v  @ Wo.T

Sharding v2: 8 cores = 4 batches x 2 head-groups (8 heads each). Every
core projects Q/K/V only for its own 512 features over the full 2048
tokens (no duplicated work anywhere -- PE row count is at the
theoretical floor of 786432 rows/core), runs attention for its 8 heads,
and o-projects its feature slice against the matching Wo rows. The two
half-outputs per batch are summed on the host (pure unshard add).

Single fused pipeline, fully transposed layout (features on partitions):
V projects first (natural layout, ones column appended so the softmax
denominator falls out of the attn@V matmul); then per head-pair: Q/K^T
projection chunks -> RoPE (DVE muls + a batched DMA partition band-swap
+ adds). K lands in TWO zero-padded stationary tiles (even head in rows
0:64 of ke, odd head in rows 64:128 of ko, other half zero via
parity-masked cos tables) so every scores matmul is a full 128-row
(128,128) PE tile against the full 128-row qtile moving operand --
avoiding the ~150ns PE reconfigure penalty that 64-row stationaries pay
on every row-size switch. Scores -> wide [128,1024] exp on ACT (scale
1/8 folded in, ACT does nothing else) -> attn@V interleaved one k-chunk
pair behind so PE fills ACT's exp latency. Normalization is
evicted-early (DVE copy frees PSUM), flushed one q-block late: exact
reciprocal runs 64-wide, gpsimd hops/broadcasts it, odd heads hop into
the o-proj operand via DMA. The o-projection for the last head-pair is
pipelined per q-block behind the final attention sweeps; output is
written bf16 and upcast host-side. All matmuls bf16, fp32 accumulation.
Nonzero biases ride an augmented K=1 contraction row (skipped when the
caller's biases are all zero).
"""

import sys

import numpy as np

try:  # concourse ships in the container; fall back to the staged repo
    import concourse.bass  # noqa: F401
except Exception:  # pragma: no cover
    sys.path.insert(0, "/opt/trn_rl_repo")

import ml_dtypes

B, S, D, H = 4, 2048, 1024, 16
HD = D // H                      # 64
P = 128
NCORES = 8
SK = S                           # 2048 tokens per core (q and k)
DO = 512                         # per-core head-group width (8 heads)
HC = 8                           # heads per core
ND = D // P                      # 8 feature contraction chunks
NPI = DO // P                    # 4 head-pair chunks
NT = SK // P                     # 16 key/token chunks
QF = 512                         # matmul moving width
NQF = SK // QF                   # 4 query blocks
ROPE_BASE = 10000.0
BF16 = ml_dtypes.bfloat16

TRACE = False                    # test harness flips this
TRACE_KW = {}
LAST = {}                        # exec_time_ns / trace path for test harness

_cache = {}


def _build_nc(with_bias):
    import concourse.bass as bass
    import concourse.mybir as mybir
    import concourse.tile as tile
    from concourse import bacc
    from contextlib import ExitStack

    f32 = mybir.dt.float32
    bf16 = mybir.dt.bfloat16
    AF = mybir.ActivationFunctionType
    PSUM = bass.MemorySpace.PSUM

    nc = bacc.Bacc(None)
    xT = nc.declare_dram_parameter("xT", [D + 1, SK], bf16, False)
    wqT = nc.declare_dram_parameter("wqT", [D + 1, DO], bf16, False)
    wkT = nc.declare_dram_parameter("wkT", [D + 1, DO], bf16, False)
    wvT = nc.declare_dram_parameter("wvT", [D + 1, DO], bf16, False)
    woT = nc.declare_dram_parameter("woT", [DO, D], bf16, False)
    cosk = nc.declare_dram_parameter("cosk", [P, SK], bf16, False)
    coske = nc.declare_dram_parameter("coske", [P, SK], bf16, False)
    cosko = nc.declare_dram_parameter("cosko", [P, SK], bf16, False)
    sink = nc.declare_dram_parameter("sink", [P, SK], bf16, False)
    out = nc.declare_dram_parameter("out", [SK, D], bf16, True)

    with tile.TileContext(nc) as tc, ExitStack() as st:
        sb = st.enter_context(tc.tile_pool(name="sb", bufs=1))
        qk = st.enter_context(tc.tile_pool(name="qk", bufs=2))
        wp = st.enter_context(tc.tile_pool(name="wp", bufs=2))
        tp = st.enter_context(tc.tile_pool(name="tp", bufs=2))
        etp = st.enter_context(tc.tile_pool(name="et", bufs=6))
        npool = st.enter_context(tc.tile_pool(name="nrm", bufs=3))
        outp = st.enter_context(tc.tile_pool(name="ou", bufs=3))
        psp = st.enter_context(tc.tile_pool(name="ps", bufs=2, space=PSUM))

        vst = [sb.tile([P, HC, HD + 1], bf16, tag=f"v{t}", name=f"v{t}")
               for t in range(NT)]
        at = [sb.tile([P, SK], bf16, tag=f"at{i}", name=f"at{i}")
              for i in range(NPI)]

        # ---- loads -----------------------------------------------------
        xs = [sb.tile([P, SK], bf16, tag=f"x{d}", name=f"x{d}")
              for d in range(ND)]
        for d_ in range(ND):
            nc.sync.dma_start(out=xs[d_][:], in_=xT[d_ * P:(d_ + 1) * P, :])
        ck = sb.tile([P, SK], bf16, tag="ck", name="ck")
        cke = sb.tile([P, SK], bf16, tag="cke", name="cke")
        cko = sb.tile([P, SK], bf16, tag="cko", name="cko")
        sk_ = sb.tile([P, SK], bf16, tag="sk", name="sk")
        nc.sync.dma_start(out=ck[:], in_=cosk[:, :])
        nc.sync.dma_start(out=cke[:], in_=coske[:, :])
        nc.sync.dma_start(out=cko[:], in_=cosko[:, :])
        nc.sync.dma_start(out=sk_[:], in_=sink[:, :])
        wv = wp.tile([P, ND, DO], bf16, tag="wv", name="wv", bufs=1)
        for d_ in range(ND):
            nc.sync.dma_start(out=wv[:, d_, :], in_=wvT[d_ * P:(d_ + 1) * P, :])
        wo = wp.tile([P, NPI, D], bf16, tag="wo", name="wo", bufs=1)
        for f in range(NPI):
            nc.sync.dma_start(out=wo[:, f, :], in_=woT[f * P:(f + 1) * P, :])
        if with_bias:
            xone = sb.tile([1, SK], bf16, tag="xone", name="xone")
            nc.sync.dma_start(out=xone[:], in_=xT[D:D + 1, :])
            wvb = wp.tile([1, DO], bf16, tag="wvb", name="wvb", bufs=1)
            nc.sync.dma_start(out=wvb[:], in_=wvT[D:D + 1, :])

        def load_wslice(wdram, wtag):
            ws = wp.tile([P, ND, P], bf16, tag=wtag, name=wtag)
            nc.sync.dma_start(
                out=ws[:],
                in_=wdram[0:D, :].rearrange("(n p) o -> p n o", p=P))
            wb = None
            if with_bias:
                wb = wp.tile([1, P], bf16, tag=wtag + "b", name=wtag + "b")
                nc.sync.dma_start(out=wb[:], in_=wdram[D:D + 1, :])
            return ws, wb

        def qk_proj(wsb, dst, dsto=None):
            """dst = rope(W[pi-slice] @ x^T + b). Q path (dsto None):
            full-width writes into dst. K path: even head -> dst rows
            0:64 (rows 64:128 stay zero via the masked cos table), odd
            head -> dsto rows 64:128 -- zero-padded 128-row stationaries
            for the scores matmuls."""
            ws, wb = wsb
            t2 = tp.tile([P, SK], bf16, tag="t2", name="t2")
            t2s = tp.tile([P, SK], bf16, tag="t2s", name="t2s")
            for c in range(SK // QF):
                ps = psp.tile([P, QF], f32, tag="pp", name="pp")
                for d_ in range(ND):
                    nc.tensor.matmul(
                        ps[:], ws[:, d_, :], xs[d_][:, c * QF:(c + 1) * QF],
                        start=(d_ == 0), stop=(not with_bias and d_ == ND - 1))
                if with_bias:
                    nc.tensor.matmul(
                        ps[:], wb[:], xone[:, c * QF:(c + 1) * QF],
                        start=False, stop=True)
                cs = slice(c * QF, (c + 1) * QF)
                if dsto is None:
                    nc.vector.tensor_mul(dst[:, cs], ps[:], ck[:, cs])
                else:
                    nc.vector.tensor_mul(dst[:, cs], ps[:], cke[:, cs])
                    nc.vector.tensor_mul(dsto[:, cs], ps[:], cko[:, cs])
                nc.vector.tensor_mul(t2[:, cs], ps[:], sk_[:, cs])
                if c % 2 == 1:
                    # band swap d<->d+32 (sync DMA queue) + add, batched
                    # over the finished 1024-wide half
                    hs_ = slice((c - 1) * QF, (c + 1) * QF)
                    for b0 in (0, 64):
                        nc.sync.dma_start(
                            out=t2s[b0:b0 + 32, hs_], in_=t2[b0 + 32:b0 + 64, hs_])
                        nc.sync.dma_start(
                            out=t2s[b0 + 32:b0 + 64, hs_], in_=t2[b0:b0 + 32, hs_])
                    if dsto is None:
                        nc.vector.tensor_add(
                            dst[:, hs_], dst[:, hs_], t2s[:, hs_])
                    else:
                        nc.vector.tensor_add(
                            dst[0:HD, hs_], dst[0:HD, hs_], t2s[0:HD, hs_])
                        nc.vector.tensor_add(
                            dsto[HD:P, hs_], dsto[HD:P, hs_], t2s[HD:P, hs_])

        # ---- V projection (natural layout, x^T stationary) -------------
        for t_ in range(NT):
            ps = psp.tile([P, DO], f32, tag="pp", name="pp")
            for d_ in range(ND):
                nc.tensor.matmul(
                    ps[:], xs[d_][:, t_ * P:(t_ + 1) * P], wv[:, d_, :],
                    start=(d_ == 0), stop=(not with_bias and d_ == ND - 1))
            if with_bias:
                nc.tensor.matmul(
                    ps[:], xone[:, t_ * P:(t_ + 1) * P], wvb[:],
                    start=False, stop=True)
            nc.vector.tensor_copy(
                vst[t_][:, :, 0:HD], ps[:].rearrange("p (h d) -> p h d", d=HD))
            nc.vector.memset(vst[t_][:, :, HD:HD + 1], 1.0)

        # ---- fused per-head-pair projection + attention ----------------
        pend = []

        def flush_one():
            # normalize in SBUF: exact reciprocal spread 64-wide (~0.3us
            # not 3.3us single-lane), gpsimd DMA-hop to p0, partition-
            # broadcast, multiply into the o-proj operand
            pi, qqs, osb_e, osb_o = pend.pop(0)
            for par, osb in ((0, osb_e), (1, osb_o)):
                smr = npool.tile([HD, 8], f32, tag="smr", name="smr")
                nc.gpsimd.dma_start(out=smr[:], in_=osb[HD:HD + 1, :])
                rcs = npool.tile([HD, 8], f32, tag="rcs", name="rcs")
                nc.vector.reciprocal(rcs[:], smr[:])
                rc = npool.tile([1, QF], f32, tag="rc", name="rc")
                nc.gpsimd.dma_start(out=rc[:], in_=rcs[:])
                bc = npool.tile([HD, QF], f32, tag="bc", name="bc")
                nc.gpsimd.partition_broadcast(bc[:], rc[:])
                if par == 0:
                    nc.vector.tensor_mul(
                        at[pi][0:HD, qqs], osb[0:HD, :], bc[:])
                else:
                    # odd heads land at partition base 64; a pure-SBUF
                    # base-shifted DVE write corrupts, so write at base
                    # 0 and DMA-hop into place
                    atm = npool.tile([HD, QF], bf16, tag="atm", name="atm")
                    nc.vector.tensor_mul(atm[:], osb[0:HD, :], bc[:])
                    nc.gpsimd.dma_start(out=at[pi][HD:P, qqs], in_=atm[:])

        def oproj(qh):
            # o-projection for one 512-wide q block; consumes the
            # transposed at[] tiles directly, writes bf16
            for qc in range(QF // P):
                qa = qh * (QF // P) + qc
                for oh in range(2):
                    ps = psp.tile([P, QF], f32, tag="pp", name="pp")
                    for f in range(NPI):
                        nc.tensor.matmul(
                            ps[:], at[f][:, qa * P:(qa + 1) * P],
                            wo[:, f, oh * QF:(oh + 1) * QF],
                            start=(f == 0), stop=(f == NPI - 1))
                    ob = outp.tile([P, QF], bf16, tag="ob", name="ob")
                    nc.vector.tensor_copy(ob[:], ps[:])
                    nc.scalar.dma_start(
                        out=out[qa * P:(qa + 1) * P, oh * QF:(oh + 1) * QF],
                        in_=ob[:])

        wnext = (load_wslice(wqT[:, 0:P], "wq"), load_wslice(wkT[:, 0:P], "wk"))
        for pi in range(NPI):
            wcur = wnext
            if pi + 1 < NPI:
                osl = slice((pi + 1) * P, (pi + 2) * P)
                wnext = (load_wslice(wqT[:, osl], "wq"),
                         load_wslice(wkT[:, osl], "wk"))
            qtile = qk.tile([P, SK], bf16, tag="qt", name="qt")
            qk_proj(wcur[0], qtile)
            ke = qk.tile([P, SK], bf16, tag="ke", name="ke")
            ko = qk.tile([P, SK], bf16, tag="ko", name="ko")
            qk_proj(wcur[1], ke, ko)

            for qh in range(NQF):
                qs = slice(qh * QF, (qh + 1) * QF)
                ope = psp.tile([HD + 1, QF], f32, tag="o", name="o")
                opo = psp.tile([HD + 1, QF], f32, tag="o", name="o")
                prev = None
                for kcp in range(NT // 2):
                    spe = psp.tile([P, 2 * QF], f32, tag="s", name="s")
                    spo = psp.tile([P, 2 * QF], f32, tag="s", name="s")
                    for j in range(2):
                        ks_ = slice((2 * kcp + j) * P, (2 * kcp + j + 1) * P)
                        js = slice(j * QF, (j + 1) * QF)
                        nc.tensor.matmul(
                            spe[:, js], ke[:, ks_], qtile[:, qs],
                            start=True, stop=True)
                        nc.tensor.matmul(
                            spo[:, js], ko[:, ks_], qtile[:, qs],
                            start=True, stop=True)
                    ee = etp.tile([P, 2 * QF], bf16, tag="e", name="e")
                    eo = etp.tile([P, 2 * QF], bf16, tag="e", name="e")
                    nc.scalar.activation(ee[:], spe[:], AF.Exp, scale=0.125)
                    nc.scalar.activation(eo[:], spo[:], AF.Exp, scale=0.125)
                    # attn@V for the previous k-chunk pair overlaps this
                    # pair's exp latency on the PE
                    if prev is not None:
                        pee, peo, pk = prev
                        for j in range(2):
                            kc = 2 * pk + j
                            js = slice(j * QF, (j + 1) * QF)
                            nc.tensor.matmul(
                                ope[:], vst[kc][:, 2 * pi, :], pee[:, js],
                                start=(kc == 0), stop=False)
                            nc.tensor.matmul(
                                opo[:], vst[kc][:, 2 * pi + 1, :], peo[:, js],
                                start=(kc == 0), stop=False)
                    prev = (ee, eo, kcp)
                pee, peo, pk = prev
                for j in range(2):
                    kc = 2 * pk + j
                    js = slice(j * QF, (j + 1) * QF)
                    nc.tensor.matmul(
                        ope[:], vst[kc][:, 2 * pi, :], pee[:, js],
                        start=False, stop=(kc == NT - 1))
                    nc.tensor.matmul(
                        opo[:], vst[kc][:, 2 * pi + 1, :], peo[:, js],
                        start=False, stop=(kc == NT - 1))

                # evict PSUM immediately (quick DVE copies free the "o"
                # slots); the reciprocal chain is flushed one q-block
                # later so it never blocks PE's PSUM evicts
                osb_e = npool.tile([HD + 1, QF], f32, tag="osb", name="osb",
                                   bufs=8)
                nc.vector.tensor_copy(osb_e[:], ope[:])
                osb_o = npool.tile([HD + 1, QF], f32, tag="osb", name="osb",
                                   bufs=8)
                nc.vector.tensor_copy(osb_o[:], opo[:])
                pend.append((pi, qs, osb_e, osb_o))
                while len(pend) > 1:
                    fpi = pend[0][0]
                    fqh = pend[0][1].start // QF
                    flush_one()
                    if fpi == NPI - 1:
                        oproj(fqh)

        while pend:
            fqh = pend[0][1].start // QF
            flush_one()
            oproj(fqh)
    nc.compile()
    return nc


def _rope_tables(pos):
    """pos [n] -> cos/sin tables [128, n] bf16: full cos, parity-masked
    cos (even rows 0:64 / odd rows 64:128, rest zero), sign-folded sin."""
    inv = ROPE_BASE ** (-np.arange(0, HD, 2, dtype=np.float64) / HD)
    fr = np.outer(pos.astype(np.float64), inv)          # [n, 32]
    c, s = np.cos(fr), np.sin(fr)
    cos64 = np.concatenate([c, c], axis=1).T            # [64, n]
    sinA = np.concatenate([s, -s], axis=1).T            # [64, n]
    z = np.zeros_like(cos64)
    ck = np.concatenate([cos64, cos64], axis=0).astype(BF16)
    cke = np.concatenate([cos64, z], axis=0).astype(BF16)
    cko = np.concatenate([z, cos64], axis=0).astype(BF16)
    sk = np.concatenate([sinA, sinA], axis=0).astype(BF16)
    return ck, cke, cko, sk


def _aug_w(w, b, g):
    """[D, D] weight + [D] bias -> bf16 [D+1, DO]: W.T columns for head
    group g, bias row appended."""
    cols = slice(g * DO, (g + 1) * DO)
    wa = np.empty((D + 1, DO), dtype=np.float32)
    wa[:D] = np.asarray(w, dtype=np.float32).T[:, cols]
    wa[D] = np.asarray(b, dtype=np.float32)[cols]
    return np.ascontiguousarray(wa).astype(BF16)


def kernel(hidden_states, position_ids, Wq, bq, Wk, bk, Wv, bv, Wo):
    from concourse import bass_utils

    with_bias = bool(
        np.any(np.asarray(bq)) or np.any(np.asarray(bk)) or np.any(np.asarray(bv)))
    key = ("nc", with_bias)
    if key not in _cache:
        _cache[key] = _build_nc(with_bias)
    nc = _cache[key]

    hs = np.asarray(hidden_states, dtype=np.float32)
    pos = np.asarray(position_ids)
    wq = [_aug_w(Wq, bq, g) for g in range(2)]
    wk = [_aug_w(Wk, bk, g) for g in range(2)]
    wv = [_aug_w(Wv, bv, g) for g in range(2)]
    woT = np.ascontiguousarray(np.asarray(Wo, dtype=np.float32).T)
    wo = [np.ascontiguousarray(woT[g * DO:(g + 1) * DO, :]).astype(BF16)
          for g in range(2)]

    xts, tabs = [], []
    for b in range(B):
        xT = np.empty((D + 1, SK), dtype=np.float32)
        xT[:D] = hs[b].T
        xT[D] = 1.0
        xts.append(np.ascontiguousarray(xT).astype(BF16))
        tabs.append(_rope_tables(np.asarray(pos[b])))

    in_maps = []
    for core in range(NCORES):
        b, g = core // 2, core % 2
        ck, cke, cko, sk = tabs[b]
        in_maps.append({
            "xT": xts[b], "wqT": wq[g], "wkT": wk[g], "wvT": wv[g],
            "woT": wo[g], "cosk": ck, "coske": cke, "cosko": cko, "sink": sk,
        })

    res = bass_utils.run_bass_kernel_spmd(
        nc, in_maps, core_ids=list(range(NCORES)), trace=TRACE, **TRACE_KW)
    LAST["exec_time_ns"] = res.exec_time_ns
    LAST["mean_exec_time_ns"] = res.mean_exec_time_ns
    LAST["trace"] = res.instructions_and_trace
    LAST["profile_json"] = res.profile_json

    outp_full = np.empty((B, S, D), dtype=np.float32)
    for b in range(B):
        outp_full[b] = (
            np.asarray(res.results[2 * b]["out"], dtype=np.float32)
            + np.asarray(res.results[2 * b + 1]["out"], dtype=np.float32))
    return outp_full
